# revision 52
# baseline (speedup 1.0000x reference)
"""Trainium2 Bass kernel for nn_NoiseReduceLayer (noisereduce non-stationary
spectral gating): STFT -> |S| -> EMA filtfilt (time) -> sigmoid gate ->
separable (65x3) smoothing conv -> mask*S -> ISTFT.

v2 rewrite of the matmul-DFT formulation:
  - Framing loads one extra (+1-shifted) column so the half-range fold
    u/v = x[n] +/- x[2048-n] comes from a shifted chunk-matrix view (xs)
    plus a partition-flipped chunk matrix (xcf, 12 small Jf matmuls)
    combined on the vector engine -- replaces 48 T-wide perm matmuls and
    16 scalar-engine PSUM evacuations per sample.
  - Reciprocal via the single-instruction DVE RECIPROCAL_APPROX_FAST
    (replaces the Ln/Exp-seeded Newton chain and its act-table loads).
  - Inverse weights resident in SBUF (no per-sample streaming); the dead
    Bi row-1024 accumulation is dropped (sin(pi*n)=0).
  - Overlap-add accumulates inside PSUM via shifted zero-padded rhs views
    of the combine perm-matmuls: one evacuation (fused *1/norm) per
    512-sample chunk group instead of per-frame copies + adds.
  - Output stored as f16 (host casts to f32); f16 transposes halve the
    PE cost of the final layout change.
  - 3-deep software pipeline: head(s) | gate-tail(s-1) | inverse(s-1).

Sharding: pure data parallel, batch 32 -> 4 samples on each of 8 cores.
"""
import numpy as np

import concourse.mybir as mybir
from concourse import bacc
from concourse.tile import TileContext
from concourse.bass_utils import run_bass_kernel_spmd

f32 = mybir.dt.float32
f16 = mybir.dt.float16
AF = mybir.ActivationFunctionType
OP = mybir.AluOpType

SR = 16000
NFFT = 2048
HOP = 512
NSAMP = 163840
T = 321        # stft frames
F = 1025       # one-sided bins
FP = 1152      # padded bins = 9*128
NCH = 324      # 512-chunks in padded signal (2 zero + 320 data + 2 zero)
BLOC = 4       # samples per core
NCORES = 8

_TF = 2.0 * SR / HOP
B_EMA = (np.sqrt(1.0 + 4.0 * _TF**2) - 1.0) / (2.0 * _TF**2)


def _build_consts():
    w = 0.5 - 0.5 * np.cos(2 * np.pi * np.arange(NFFT) / NFFT)
    k = np.arange(FP)
    n = np.arange(1, 1025)                    # half range (w[0]=0 kills n=0)
    ang = 2 * np.pi * np.outer(k, n) / NFFT   # (FP, 1024)
    half = np.ones(1024)
    half[-1] = 0.5                            # n=1024 self-pairs in u
    CrH = np.cos(ang) * w[None, 1:1025] * half[None, :]
    CiH = -np.sin(ang) * w[None, 1:1025]
    CrH[F:, :] = 0.0
    CiH[F:, :] = 0.0
    ck = np.ones(FP)
    ck[0] = 0.5
    ck[1024] = 0.5
    ck[F:] = 0.0
    BrH = (2.0 / NFFT) * np.cos(ang) * ck[:, None] * w[None, 1:1025]   # (FP, 1024)
    BiH = -(2.0 / NFFT) * np.sin(ang) * ck[:, None] * w[None, 1:1025]

    b = B_EMA
    L = np.zeros((T, T))
    for t in range(T):
        L[t, 1:t + 1] = b * (1 - b) ** (t - np.arange(1, t + 1))
        L[t, 0] = b * (1 - b) ** t + (1 - b) ** (t + 1)
    Pf = np.eye(T)[::-1]
    M = Pf @ L @ Pf @ L

    fpart = np.concatenate([
        np.linspace(0, 1, 33, endpoint=False),
        np.linspace(1, 0, 34),
    ])[1:-1]
    Gf = np.zeros((FP, FP))
    for fo in range(F):
        lo = max(0, fo - 32)
        hi = min(F, fo + 33)
        Gf[lo:hi, fo] = fpart[lo - fo + 32:hi - fo + 32] / 66.0

    npad = NSAMP + NFFT
    norm = np.zeros(npad)
    for t in range(T):
        norm[t * HOP:t * HOP + NFFT] += w**2
    norm = np.where(norm > 1e-10, norm, 1.0)
    rnorm = 1.0 / norm

    # --- device layouts (stationary matrices pre-transposed) ---
    # fwd stationary (128, 8, 1152): [p, a, f] = C*H[f, n], n = 128a+p+1
    WrT = np.zeros((128, 8, FP), np.float16)
    WiT = np.zeros((128, 8, FP), np.float16)
    for a in range(8):
        WrT[:, a, :] = CrH[:, 128 * a:128 * a + 128].T
        WiT[:, a, :] = CiH[:, 128 * a:128 * a + 128].T
    # inverse stationary (128, 9, 1024): [p, kt, n-1] = B*H[128kt+p, n]
    BrT = np.zeros((128, 9, 1024), np.float16)
    BiT = np.zeros((128, 9, 1024), np.float16)
    for kt in range(9):
        BrT[:, kt, :] = BrH[128 * kt:128 * kt + 128, :]
        BiT[:, kt, :] = BiH[128 * kt:128 * kt + 128, :]
    # EMA stationary (128, 3, 384): [p, st, t'] = M[t', 128st+p]
    MT = np.zeros((128, 3, 384), np.float16)
    for st in range(3):
        s0 = 128 * st
        ns = min(128, T - s0)
        MT[:ns, st, :T] = M[:, s0:s0 + ns].T
    # freq-conv stationary (128, 9, 3, 128)
    GT = np.zeros((128, 9, 3, 128), np.float16)
    for fot in range(9):
        for ix in range(3):
            fit = fot + ix - 1
            if 0 <= fit < 9:
                GT[:, fot, ix, :] = Gf[128 * fit:128 * fit + 128,
                                       128 * fot:128 * fot + 128]
    # OLA reciprocal norm (128, 4, 324): [p, r, j] = rnorm[512j+128r+p]
    RN = np.zeros((128, 4, NCH), np.float32)
    for r in range(4):
        for p in range(128):
            RN[p, r, :] = rnorm[np.arange(NCH) * 512 + 128 * r + p]

    # permutation matrices as lhsT[k_in, m_out] (out[m] = sum_k P[k,m]*in[k])
    def perm(fn, sign=1.0):
        Pm = np.zeros((128, 128), np.float16)
        for m in range(128):
            kk = fn(m)
            if kk is not None:
                Pm[kk, m] = sign
        return Pm
    Jf = perm(lambda m: 127 - m)                         # out[m] = in[127-m]
    S_dn = perm(lambda m: m - 1 if m >= 1 else None)     # out[m] = in[m-1]
    S_cn = perm(lambda m: 127 if m == 0 else None)       # out[0] = in[127]
    PERMS = np.stack([Jf, S_dn, S_cn], axis=1)           # (128, 3, 128)
    return WrT, WiT, BrT, BiT, MT, GT, RN, PERMS


def _register_const(nc, dtype, value):
    t = nc.alloc_sbuf_tensor(f"const-{dtype.name}-{value}", [128, 1], dtype)
    nc.gpsimd.memset(t.ap(), value)
    nc.const_aps.aps[(dtype, value)] = t.ap()


# perm indices in the pm tile
P_JF, P_SDN, P_SCN = range(3)
EMA_BIAS = 1e-6


def _build_nc():
    WrT, WiT, BrT, BiT, MT, GT, RN, PERMS = _build_consts()

    nc = bacc.Bacc("TRN2", target_bir_lowering=False)
    _register_const(nc, f32, -30.0)
    _register_const(nc, f32, EMA_BIAS)
    nc.all_engine_barrier()

    x = nc.dram_tensor("x", [BLOC, NSAMP], f32, kind="ExternalInput")
    y = nc.dram_tensor("y", [BLOC, NSAMP], f16, kind="ExternalOutput")
    dWr = nc.inline_tensor(WrT, name="dWr")
    dWi = nc.inline_tensor(WiT, name="dWi")
    dBr = nc.inline_tensor(BrT, name="dBr")
    dBi = nc.inline_tensor(BiT, name="dBi")
    dMT = nc.inline_tensor(MT, name="dMT")
    dGT = nc.inline_tensor(GT, name="dGT")
    dRN = nc.inline_tensor(RN, name="dRN")
    dPm = nc.inline_tensor(PERMS, name="dPm")
    dI16 = nc.inline_tensor(np.eye(128, dtype=np.float16), name="dI16")

    xv = x.ap().rearrange("b (j c) -> b j c", c=512)   # (4, 320, 512)
    yv = y.ap().rearrange("b (j c) -> b j c", c=512)

    with TileContext(nc) as tc:
        with tc.tile_pool(name="cst", bufs=1) as cp:
            idt16 = cp.tile([128, 128], f16)
            nc.sync.dma_start(out=idt16[:, :], in_=dI16.ap()[:, :])
            pm = cp.tile([128, 3, 128], f16)
            nc.sync.dma_start(out=pm[:, :, :], in_=dPm.ap()[:, :, :])
            mt = cp.tile([128, 3, 384], f16)
            gt = cp.tile([128, 9, 3, 128], f16)
            rn = cp.tile([128, 4, NCH], f32)
            wrh = cp.tile([128, 8, FP], f16)
            wih = cp.tile([128, 8, FP], f16)

            with tc.tile_pool(name="dat", bufs=1) as dp:
                _pipeline(nc, tc, dp, xv, yv, dBr, dBi, dWr, dWi, dMT, dGT, dRN,
                          idt16, pm, mt, gt, rn, wrh, wih)

    nc.finalize()
    return nc


def _pipeline(nc, tc, dp, xv, yv, dBr, dBi, dWr, dWi, dMT, dGT, dRN,
              idt16, pm, mt, gt, rn, wrh, wih):
    # ---- persistent tiles ----
    sr = [dp.tile([128, 9, T], f16, name=f"sr{i}", tag=f"sr{i}") for i in range(2)]
    si = [dp.tile([128, 9, T], f16, name=f"si{i}", tag=f"si{i}") for i in range(2)]
    ratio = [dp.tile([128, 3, FP], f16, name=f"rat{i}", tag=f"rat{i}")
             for i in range(2)]
    at = dp.tile([128, 3, FP], f16, name="at", tag="at")
    smt3 = dp.tile([128, 3, FP], f32, name="smt3", tag="smt3")
    rscr = dp.tile([128, FP], f32, name="rscr", tag="rscr")
    ut = dp.tile([128, 8, T], f16, name="ut", tag="ut")
    vt = dp.tile([128, 8, T], f16, name="vt", tag="vt")
    xcf = dp.tile([128, 4, NCH], f16, name="xcf", tag="xcf")
    msk = dp.tile([128, 9, T + 2], f16, name="msk", tag="msk")
    mtc = dp.tile([128, 9, T], f16, name="mtc", tag="mtc")
    yp = dp.tile([128, 8, T + 7], f16, name="yp", tag="yp")
    ym = dp.tile([128, 8, T + 7], f16, name="ym", tag="ym")
    ob = dp.tile([128, 3, 4, 128], f16, name="ob", tag="ob")
    warm = dp.tile([32, 8], f16, name="warm", tag="warm")
    # inverse weights resident (loaded once, overlapped with sample-0 head)
    brh = dp.tile([128, 9, 1024], f16, name="brh", tag="brh")
    bih = dp.tile([128, 9, 1024], f16, name="bih", tag="bih")

    # one-time zero pads; at rows 65..127 of the tt=2 block are never
    # written by the absT evacuation but ARE read by the EMA contraction
    # (zero weights there -- must not be NaN garbage)
    nc.vector.memset(at[64:128, 2, :], 0.0)
    nc.vector.memset(smt3[:, :, F:FP], 1.0)
    nc.vector.memset(msk[:, :, 0:1], 0.0)
    nc.vector.memset(msk[:, :, T + 1:T + 2], 0.0)
    nc.vector.memset(yp[:, :, 0:3], 0.0)
    nc.vector.memset(yp[:, :, T + 3:T + 7], 0.0)
    nc.vector.memset(ym[:, :, 0:3], 0.0)
    nc.vector.memset(ym[:, :, T + 3:T + 7], 0.0)

    # ---- PSUM pools ----
    ptp = tc.alloc_tile_pool(name="ptp", bufs=2, space="PSUM")    # transposes
    p321 = tc.alloc_tile_pool(name="p321", bufs=3, space="PSUM")  # [128,T] f32
    pmx = tc.alloc_tile_pool(name="pmx", bufs=3, space="PSUM")    # ratioT/out/OLA

    def head(s):
        sl = s % 2
        # ---- framing: xr rows of 513 (incl +1 col for the shifted view) ----
        xcs = dp.tile([128, 2, 4, NCH], f16, name="xcs", tag="xcs", bufs=2)
        for jt in range(3):
            xr16 = dp.tile([128, 513], f16, name="xr16", tag="xr16", bufs=2)
            if jt == 0:
                nc.vector.memset(xr16[0:2, :], 0.0)
                nc.gpsimd.dma_start(out=xr16[2:128, 0:512], in_=xv[s, 0:126, :])
                nc.gpsimd.dma_start(out=xr16[1:128, 512:513], in_=xv[s, 0:127, 0:1])
            elif jt == 1:
                nc.gpsimd.dma_start(out=xr16[:, 0:512], in_=xv[s, 126:254, :])
                nc.gpsimd.dma_start(out=xr16[:, 512:513], in_=xv[s, 127:255, 0:1])
            else:
                nc.vector.memset(xr16[64:128, :], 0.0)
                nc.gpsimd.dma_start(out=xr16[0:66, 0:512], in_=xv[s, 254:320, :])
                nc.gpsimd.dma_start(out=xr16[0:65, 512:513], in_=xv[s, 255:320, 0:1])
            ptg = ptp.tile([128, 2, 4, 128], f16, name="ptg", tag="ptg", bufs=2)
            for mtl in range(4):
                nc.tensor.transpose(ptg[:, 0, mtl, :],
                                    xr16[:, 128 * mtl:128 * mtl + 128], idt16[:, :])
                nc.tensor.transpose(ptg[:, 1, mtl, :],
                                    xr16[:, 128 * mtl + 1:128 * mtl + 129], idt16[:, :])
            nj = 128 if jt < 2 else NCH - 256
            nc.vector.tensor_copy(out=xcs[:, :, :, 128 * jt:128 * jt + nj],
                                  in_=ptg[:, :, :, 0:nj])

        # ---- partition flip: xcf[p,mt,c] = xpad[512c + 511-128mt-p] ----
        for mtl in range(4):
            pxf = p321.tile([128, NCH], f32, name="pxf", tag="p321", bufs=3)
            for jt in range(3):
                nj = 128 if jt < 2 else NCH - 256
                nc.tensor.matmul(pxf[:, 128 * jt:128 * jt + nj], pm[:, P_JF, :],
                                 xcs[:, 0, 3 - mtl, 128 * jt:128 * jt + nj],
                                 start=True, stop=True)
            nc.scalar.copy(out=xcf[:, mtl, :], in_=pxf[:, :])

        # ---- fold on DVE: u/v = xs +/- xcf views ----
        for a in range(8):
            bb = 15 - a
            fmt = 3 - bb % 4
            fc0 = bb // 4
            xsv = xcs[:, 1, a % 4, a // 4:a // 4 + T]
            xfv = xcf[:, fmt, fc0:fc0 + T]
            nc.vector.tensor_tensor(out=ut[:, a, :], in0=xsv, in1=xfv, op=OP.add)
            nc.gpsimd.tensor_tensor(out=vt[:, a, :], in0=xsv, in1=xfv, op=OP.subtract)

        return None

    def head_fwd(s):
        sl = s % 2
        # ---- forward DFT (half-range contraction) + |S| ----
        ab = {}
        for ft in range(9):
            pr = p321.tile([128, T], f32, name="pr", tag="p321", bufs=3)
            pi = p321.tile([128, T], f32, name="pi", tag="p321", bufs=3)
            for a in range(8):
                nc.tensor.matmul(pr[:, :], wrh[:, a, 128 * ft:128 * ft + 128],
                                 ut[:, a, :], start=(a == 0), stop=(a == 7))
            for a in range(8):
                nc.tensor.matmul(pi[:, :], wih[:, a, 128 * ft:128 * ft + 128],
                                 vt[:, a, :], start=(a == 0), stop=(a == 7))
            nc.scalar.copy(out=sr[sl][:, ft, :], in_=pr[:, :])
            nc.scalar.copy(out=si[sl][:, ft, :], in_=pi[:, :])
            # |S|^2 from the f16 copies (f16 keeps DVE in fast mode)
            sq = dp.tile([128, T], f16, name="sq", tag="sq", bufs=1)
            nc.gpsimd.tensor_tensor(out=sq[:, :], in0=sr[sl][:, ft, :],
                                    in1=sr[sl][:, ft, :], op=OP.mult)
            sq2 = dp.tile([128, T], f16, name="sq2", tag="sq2", bufs=1)
            nc.vector.tensor_tensor(out=sq2[:, :], in0=si[sl][:, ft, :],
                                    in1=si[sl][:, ft, :], op=OP.mult)
            nc.gpsimd.tensor_tensor(out=sq[:, :], in0=sq[:, :], in1=sq2[:, :],
                                    op=OP.add)
            abt = dp.tile([128, T], f16, name="ab", tag=f"ab{ft}")
            nc.scalar.sqrt(out=abt[:, :], in_=sq[:, :])
            ab[ft] = abt

        return ab

    def head_gate(s, ab):
        sl = s % 2
        # prewarm the Sqrt table during fwd slack
        nc.scalar.activation(out=warm[:, :], in_=idt16[0:32, 0:8], func=AF.Sqrt)
        # ---- transpose |S| -> at (t, f) ----
        for tt in range(3):
            cols = 128 if tt < 2 else T - 256
            for fg in range(3):
                pta = ptp.tile([128, 2, 4, 128], f16, name="pta", tag="ptg", bufs=2)
                for i in range(3):
                    ft = 3 * fg + i
                    nc.tensor.transpose(pta[0:cols, 0, i, :],
                                        ab[ft][:, 128 * tt:128 * tt + cols],
                                        idt16[:, :])
                nc.vector.tensor_copy(out=at[0:cols, tt, 384 * fg:384 * fg + 384],
                                      in_=pta[0:cols, 0, 0:3, :])

        # ---- EMA smooth (dense matmul over frames) ----
        for tt in range(3):
            for fc in range(3):
                nf = 384 if fc < 2 else F - 768
                pe = p321.tile([128, 384], f32, name="pema", tag="p321", bufs=3)
                for st in range(3):
                    nc.tensor.matmul(pe[:, 0:nf], mt[:, st, 128 * tt:128 * tt + 128],
                                     at[:, st, 384 * fc:384 * fc + nf],
                                     start=(st == 0), stop=(st == 2))
                nc.vector.tensor_scalar_add(out=smt3[:, tt, 384 * fc:384 * fc + nf],
                                            in0=pe[:, 0:nf], scalar1=EMA_BIAS)
        return None

    def recip_ratio(s):
        sl = s % 2
        # ---- reciprocal (single DVE op) + ratio ----
        for tt in range(3):
            nc.vector.reciprocal_approx_fast(out=rscr[:, :], in_=smt3[:, tt, :])
            nc.vector.tensor_tensor(out=ratio[sl][:, tt, :], in0=at[:, tt, :],
                                    in1=rscr[:, :], op=OP.mult)

    def ratio_sig(s):
        sl = s % 2
        # ---- transpose ratio -> (f, t); sigmoid( 10*x - 30 ) ----
        for ft in range(9):
            ptr = pmx.tile([128, 384], f16, name="ptr", tag="mx", bufs=3)
            for tt in range(3):
                cols = 128 if tt < 2 else T - 256
                nc.tensor.transpose(ptr[:, 128 * tt:128 * tt + cols],
                                    ratio[sl][0:cols, tt, 128 * ft:128 * ft + 128],
                                    idt16[0:cols, 0:cols])
            nc.scalar.activation(out=msk[:, ft, 1:1 + T], in_=ptr[:, 0:T],
                                 func=AF.Sigmoid, scale=10.0, bias=-30.0)
        return None

    def conv_mask(s):
        sl = s % 2
        # ---- 3-tap time conv [0.5, 1, 0.5] (chunked to unblock conv) ----
        for f0, f1 in ((0, 1), (1, 3), (3, 5), (5, 7), (7, 9)):
            fs = slice(f0, f1)
            nc.vector.tensor_tensor(out=mtc[:, fs, :], in0=msk[:, fs, 0:T],
                                    in1=msk[:, fs, 2:T + 2], op=OP.add)
            nc.vector.scalar_tensor_tensor(out=mtc[:, fs, :], in0=mtc[:, fs, :],
                                           scalar=0.5, in1=msk[:, fs, 1:T + 1],
                                           op0=OP.mult, op1=OP.add)
        # ---- banded freq conv (matmul) + S * mask in place ----
        for fot in range(9):
            pmn = p321.tile([128, T], f32, name="pmn", tag="p321", bufs=3)
            ixs = [ix for ix in range(3) if 0 <= fot + ix - 1 < 9]
            for i, ix in enumerate(ixs):
                nc.tensor.matmul(pmn[:, :], gt[:, fot, ix, :], mtc[:, fot + ix - 1, :],
                                 start=(i == 0), stop=(i == len(ixs) - 1))
            mkf = dp.tile([128, T], f16, name="mkf", tag="mkf", bufs=1)
            nc.scalar.copy(out=mkf[:, :], in_=pmn[:, :])
            nc.gpsimd.tensor_tensor(out=sr[sl][:, fot, :], in0=sr[sl][:, fot, :],
                                    in1=mkf[:, :], op=OP.mult)
            nc.gpsimd.tensor_tensor(out=si[sl][:, fot, :], in0=si[sl][:, fot, :],
                                    in1=mkf[:, :], op=OP.mult)

        return None

    def inv(s):
        sl = s % 2
        # ---- half-range inverse DFT: yp/ym over n = 128a+p+1 ----
        for a in range(8):
            pp = p321.tile([128, T], f32, name="pp", tag="p321", bufs=3)
            for kt in range(9):
                nc.tensor.matmul(pp[:, :], brh[:, kt, 128 * a:128 * a + 128],
                                 sr[sl][:, kt, :], start=(kt == 0), stop=(kt == 8))
            pq = p321.tile([128, T], f32, name="pq", tag="p321", bufs=3)
            for kt in range(8):    # Bi row 1024 is zero -> kt=8 dropped
                nc.tensor.matmul(pq[:, :], bih[:, kt, 128 * a:128 * a + 128],
                                 si[sl][:, kt, :], start=(kt == 0), stop=(kt == 7))
            pbf = dp.tile([128, T], f16, name="pbf", tag="pbf", bufs=1)
            nc.scalar.copy(out=pbf[:, :], in_=pp[:, :])
            pqf = dp.tile([128, T], f16, name="pqf", tag="pqf", bufs=1)
            nc.vector.tensor_copy(out=pqf[:, :], in_=pq[:, :])
            nc.gpsimd.tensor_tensor(out=yp[:, a, 3:3 + T], in0=pbf[:, :],
                                    in1=pqf[:, :], op=OP.add)
            nc.gpsimd.tensor_tensor(out=ym[:, a, 3:3 + T], in0=pbf[:, :],
                                    in1=pqf[:, :], op=OP.subtract)

        return None

    def combine(s):
        # prewarm the Sigmoid table for the next ratio_sig during slack
        nc.scalar.activation(out=warm[:, :], in_=idt16[0:32, 0:8], func=AF.Sigmoid)
        # ---- combine + overlap-add accumulated in PSUM; output
        # transposes run one chunk-group behind the accumulations ----
        def emit_pto(rr, acf):
            pto = pmx.tile([128, 3, 128], f16, name="pto", tag="mx", bufs=3)
            for jt in range(3):
                cj = 128 if jt < 2 else 64
                nc.tensor.transpose(pto[0:cj, jt, :],
                                    acf[:, 2 + 128 * jt:2 + 128 * jt + cj],
                                    idt16[:, :])
            nc.vector.tensor_copy(out=ob[:, :, rr, :], in_=pto[:, :, :])

        prev = None
        for rr in range(4):
            acc = pmx.tile([128, NCH], f32, name="acc", tag="mx", bufs=3)
            mms = []
            for d in range(4):
                nt = 4 * d + rr
                if nt <= 7:
                    mms.append((P_SDN, yp, nt, d))
                    if nt > 0:
                        mms.append((P_SCN, yp, nt - 1, d))
                else:
                    mms.append((P_JF, ym, 15 - nt, d))
            for i, (pidx, src, a, d) in enumerate(mms):
                nc.tensor.matmul(acc[:, :], pm[:, pidx, :],
                                 src[:, a, 3 - d:3 - d + NCH],
                                 start=(i == 0), stop=(i == len(mms) - 1))
            acf = dp.tile([128, NCH], f16, name="acf", tag="acf", bufs=1)
            nc.vector.tensor_tensor(out=acf[:, :], in0=acc[:, :], in1=rn[:, rr, :],
                                    op=OP.mult)
            if prev is not None:
                emit_pto(*prev)
            prev = (rr, acf)
        emit_pto(*prev)
        # one store per 128-row block, all 4 chunk groups at once
        nc.sync.dma_start(out=yv[s, 0:128, :], in_=ob[:, 0, :, :])
        nc.sync.dma_start(out=yv[s, 128:256, :], in_=ob[:, 1, :, :])
        nc.sync.dma_start(out=yv[s, 256:320, :], in_=ob[0:64, 2, :, :])

    # ---- software pipeline, fine-grained interleave ----
    head(0)
    # big consts stream in behind sample 0's input loads
    for c0, c1 in ((0, 256), (256, 512), (512, 832), (832, FP)):
        nc.sync.dma_start(out=wrh[:, :, c0:c1], in_=dWr.ap()[:, :, c0:c1])
        nc.sync.dma_start(out=wih[:, :, c0:c1], in_=dWi.ap()[:, :, c0:c1])
    nc.sync.dma_start(out=mt[:, :, :], in_=dMT.ap()[:, :, :])
    ab0 = head_fwd(0)
    nc.sync.dma_start(out=brh[:, :, :], in_=dBr.ap()[:, :, :])
    nc.sync.dma_start(out=bih[:, :, :], in_=dBi.ap()[:, :, :])
    nc.sync.dma_start(out=gt[:, :, :, :], in_=dGT.ap()[:, :, :, :])
    nc.sync.dma_start(out=rn[:, :, :], in_=dRN.ap()[:, :, :])
    head(1)
    head_gate(0, ab0)
    for s in range(1, BLOC):
        ab = head_fwd(s)
        if s == 1:
            recip_ratio(0)
        ratio_sig(s - 1)
        head_gate(s, ab)
        conv_mask(s - 1)
        if s + 1 < BLOC:
            head(s + 1)
        if s >= 2:
            combine(s - 2)
        inv(s - 1)
        recip_ratio(s)
    ratio_sig(BLOC - 1)
    conv_mask(BLOC - 1)
    combine(BLOC - 2)
    inv(BLOC - 1)
    combine(BLOC - 1)

    for p in (pmx, p321, ptp):
        p.release()


_NC = None


def _get_nc():
    global _NC
    if _NC is None:
        _NC = _build_nc()
    return _NC


def _run(x, trace=False):
    nc = _get_nc()
    x = np.ascontiguousarray(np.asarray(x), np.float32)
    assert x.shape == (NCORES * BLOC, NSAMP)
    in_maps = [{"x": x[BLOC * i:BLOC * i + BLOC]} for i in range(NCORES)]
    res = run_bass_kernel_spmd(nc, in_maps, list(range(NCORES)), trace=trace)
    out = np.concatenate([np.asarray(res.results[i]["y"], np.float32)
                          for i in range(NCORES)], axis=0)
    return out, res


def kernel(x):
    out, _ = _run(x)
    return out
